# revision 27
# baseline (speedup 1.0000x reference)
"""AttentionAugmentedConv2d Trainium2 kernel (8 NeuronCores, SPMD).

Sharding: core c -> image b = c//2, half g = c%2.
Each core computes, for its image:
  - conv3x3 for 128 of the 256 conv_out channels
  - qkv conv3x3 for its 4 heads (128 q + 128 k + 128 v channels)
  - full attention (with relative position logits) for those 4 heads
  - a partial 1x1 "combine" conv: all 256 output channels contracted over
    its 128 att channels.  Host sums the two halves per image (att bias is
    folded into the g==0 core's bias input).

All matmuls run in bf16 (fp32 PSUM accumulation; end-to-end ~4e-3
rel-absmax error) at full PE rate with fast weight loads.
Relative logits are folded into the QK matmul by augmenting the
contraction dim: logits^T = [k; Ex; Ey]^T @ [q; RX^T; RY^T], with Ex/Ey
constant 0/1 indicator matrices and RX/RY produced by a small q @ kr^T
matmul followed by a shear-gather DMA through DRAM (the rel_to_abs skew
is a linear access pattern in (y, x, c) coordinates).

Logits are computed transposed (keys on partitions, queries free) so that
softmax needs no transpose: exp (constant -20 bias, cancels in
normalization), denominators via a ones-column interleaved into fv^T.
The reference's raw reshape of att (B,NH,HW,dvh)->(B,NH,dvh,H,W) is a
flat memory reinterpretation, obtained for free via a DRAM roundtrip.

Schedule: warmup matmuls keep HAM warm during input DMAs; all shear
gathers are issued in the prologue; heads are software-pipelined (head
h's compute interleaves head h+1's aug-transposes and head h-1's output
normalization) so the PE never sees a low-density window.
"""

import numpy as np
import ml_dtypes

BF = ml_dtypes.bfloat16

NH = 8
DK = 256
DV = 256
DKH = DK // NH          # 32
DVH = DV // NH          # 32
H = W = 32
HW = H * W              # 1024
B = 4
CIN = 128
COUT = 512
KS = 3
NCORES = 8
HPC = NH // 2           # heads per core = 4
PADW = W + 2            # 34
NPIX = PADW * PADW      # 1156
RELW = 2 * W - 1        # 63
RELC = 2 * RELW         # 126
PRODW = HPC * RELC      # 504

_CACHE = {}


def _build_bass():
    import concourse.bass as bass
    import concourse.mybir as mybir
    from concourse import bacc
    from concourse.tile import TileContext
    from concourse.masks import make_identity

    dt = mybir.dt
    f32 = dt.float32
    bf16 = dt.bfloat16
    AF = mybir.ActivationFunctionType
    MUL = mybir.AluOpType.mult

    nc = bacc.Bacc("TRN2", target_bir_lowering=False, debug=False,
                   num_devices=NCORES)

    # ---- I/O ----
    xp_d = nc.dram_tensor("xp", [CIN, NPIX], bf16, kind="ExternalInput")
    wall_d = nc.dram_tensor("wall", [CIN, 4 * 9 * 128], bf16, kind="ExternalInput")
    ball_d = nc.dram_tensor("ball", [4, 128], f32, kind="ExternalInput")
    krbd_d = nc.dram_tensor("krbd", [128, PRODW], bf16, kind="ExternalInput")
    exey_d = nc.dram_tensor("exey", [64, HW], bf16, kind="ExternalInput")
    awt_d = nc.dram_tensor("awt", [128, 256], bf16, kind="ExternalInput")
    ab_d = nc.dram_tensor("ab", [2, 128], f32, kind="ExternalInput")
    oc_d = nc.dram_tensor("oc", [128, HW], f32, kind="ExternalOutput")
    oa_d = nc.dram_tensor("oa", [256, HW], f32, kind="ExternalOutput")
    prod_d = nc.dram_tensor("prodd", [HW, PRODW], bf16)
    att_d = nc.dram_tensor("attd", [128, HW], bf16)

    with TileContext(nc) as tc:
        with (
            tc.tile_pool(name="consts", bufs=1) as consts,
            tc.tile_pool(name="pers", bufs=1) as pers,
            tc.tile_pool(name="expp", bufs=3) as expp,
            tc.tile_pool(name="work", bufs=3) as work,
            tc.tile_pool(name="outp", bufs=2) as outp,
            tc.tile_pool(name="psB", bufs=3, space="PSUM") as psB,
            tc.tile_pool(name="psT", bufs=2, space="PSUM") as psT,
        ):
            # ---------- constants / identities (warmup chain first) ----------
            warm_f = consts.tile([128, 512], f32)
            nc.gpsimd.memset(warm_f[:], 0.5)
            warm_r = consts.tile([128, 512], bf16)
            nc.vector.tensor_copy(warm_r[:], warm_f[:])
            ident = consts.tile([128, 128], f32)
            make_identity(nc, ident[:])
            identb = consts.tile([128, 128], bf16)
            nc.vector.tensor_copy(identb[:], ident[:])
            ones_f = consts.tile([128, 1], f32)
            nc.gpsimd.memset(ones_f[:], 1.0)
            ones_r = consts.tile([128, 1], bf16)
            nc.vector.tensor_copy(ones_r[:], ones_f[:])
            negc = consts.tile([128, 1], f32)
            nc.gpsimd.memset(negc[:], -20.0)

            # ---------- input DMAs (ordered by first use) ----------
            xp_sb = consts.tile([CIN, NPIX], bf16)
            nc.sync.dma_start(xp_sb[:], xp_d[:])
            wall_sb = [consts.tile([CIN, 9, 128], bf16, tag=f"wall{g}", name=f"wall{g}")
                       for g in range(4)]
            wd4 = wall_d[:].rearrange("c (g t o) -> c g t o", g=4, t=9)
            for g in (1, 2, 3, 0):   # q, k, v, conv order
                nc.sync.dma_start(wall_sb[g][:], wd4[:, g])
            ball_sb = consts.tile([128, 4], f32)
            nc.sync.dma_start(ball_sb[:], ball_d[:].rearrange("g c -> c g"))
            krbd_sb = consts.tile([128, PRODW], bf16)
            nc.sync.dma_start(krbd_sb[:], krbd_d[:])
            exey_sb = consts.tile([64, HW], bf16)
            nc.scalar.dma_start(exey_sb[:], exey_d[:])
            awt_sb = consts.tile([128, 256], bf16)
            nc.scalar.dma_start(awt_sb[:], awt_d[:])
            ab_sb = consts.tile([128, 2], f32)
            nc.scalar.dma_start(ab_sb[:], ab_d[:].rearrange("g c -> c g"))

            # ---------- HAM warmup (runs while DMAs land) ----------
            for wi in range(16):
                wps = psB.tile([128, 1024], f32, tag="big", name="psb")[:, 0:512]
                nc.tensor.matmul(wps[:], warm_r[:, 0:128], warm_r[:],
                                 start=True, stop=True)

            # ---------- persistent tiles ----------
            q_all = pers.tile([128, HW], bf16, tag="q")
            k_all = pers.tile([128, HW], bf16, tag="k")
            v_all = pers.tile([128, HW], bf16, tag="v")
            vt_all = pers.tile([128, 8, 34 * HPC], bf16, tag="vt")
            qaug = [pers.tile([96, HW], bf16, tag=f"qaug{h}", name=f"qaug{h}") for h in range(HPC)]
            kaug = [pers.tile([96, HW], bf16, tag=f"kaug{h}", name=f"kaug{h}") for h in range(HPC)]
            rxa = [pers.tile([128, 8, 32], bf16, tag=f"rxa{h}", name=f"rxa{h}") for h in range(HPC)]
            rya = [pers.tile([128, 8, 32], bf16, tag=f"rya{h}", name=f"rya{h}") for h in range(HPC)]
            attn_sb = [pers.tile([33, HW], bf16, tag=f"attn{h}", name=f"attn{h}")
                       for h in range(2)]
            att_hd = [pers.tile([128, 8, 32], bf16, tag=f"ahd{h}", name=f"ahd{h}")
                      for h in range(2)]
            attr_sb = pers.tile([128, HW], bf16, tag="attr")

            # ones columns interleaved into vt_all (col 32 of each 34-block)
            vt4 = vt_all[:].rearrange("p a (h c) -> p a h c", h=HPC)
            nc.vector.tensor_copy(
                vt4[:, :, :, 32:33],
                ones_r[:].unsqueeze(1).unsqueeze(1).to_broadcast((128, 8, HPC, 1)))

            # ---------- convs ----------
            xp3 = xp_sb[:].rearrange("c (a b) -> c a b", a=PADW)

            def conv_mm(grp, half, ps3, taps):
                for tap in taps:
                    dy, dx = tap // 3, tap % 3
                    rhs = xp3[:, 16 * half + dy:16 * half + dy + 16,
                              dx:dx + W]
                    nc.tensor.matmul(ps3, wall_sb[grp][:, tap, :], rhs,
                                     start=(tap == 0), stop=(tap == 8))

            def conv_group(grp, dst):
                for half in range(2):
                    ps = psB.tile([128, 1024], f32, tag="big", name="psb")[:, 0:512]
                    conv_mm(grp, half, ps[:].rearrange("p (a b) -> p a b", a=16),
                            range(9))
                    bias = ball_sb[:, grp:grp + 1]
                    if dst is None:
                        ocs = outp.tile([128, 512], f32, tag="ocs")
                        nc.scalar.activation(ocs[:], ps[:], AF.Identity,
                                             bias=bias)
                        nc.sync.dma_start(
                            oc_d[:, half * 512:(half + 1) * 512], ocs[:])
                    else:
                        nc.scalar.activation(
                            dst[:, half * 512:(half + 1) * 512], ps[:],
                            AF.Identity, bias=bias)

            def shear_one(h, ic, eng, rx_only=False, ry_only=False):
                if not ry_only:
                    # RX[(yl,xI), c] = prod[(4ic+yl)*32+xI, h*126 + c+31-xI]
                    off_x = h * RELC + 31 + ic * 128 * PRODW
                    src_x = bass.AP(prod_d, off_x,
                                    [[32 * PRODW, 4], [PRODW - 1, 32], [1, 32]])
                    eng.dma_start(rxa[h][:, ic, :], src_x)
                if not rx_only:
                    # RY[(yl,xI), r] = prod[xI*32+4ic+yl, h*126+63 + r+31-xI]
                    off_y = h * RELC + 63 + 31 + ic * 4 * PRODW
                    src_y = bass.AP(prod_d, off_y,
                                    [[PRODW, 4], [32 * PRODW - 1, 32], [1, 32]])
                    eng.dma_start(rya[h][:, ic, :], src_y)

            def shear_dmas(h):
                eng = {0: nc.sync, 1: nc.gpsimd, 2: nc.gpsimd, 3: nc.sync}[h]
                for ic in range(8):
                    shear_one(h, ic, eng)

            conv_group(1, q_all)

            def emit_prod(ic):
                # rel prod (depends only on q); psT slots so conv psums flow
                pp = psT.tile([128, 512], f32, tag="t", name="pp")[:, 0:PRODW]
                nc.tensor.matmul(pp[:], q_all[:, ic * 128:(ic + 1) * 128],
                                 krbd_sb[:], start=True, stop=True)
                psb = work.tile([128, PRODW], bf16, tag="prodsb")
                if ic % 2 == 0:
                    nc.vector.tensor_copy(psb[:], pp[:])
                else:
                    nc.scalar.activation(psb[:], pp[:], AF.Identity)
                nc.sync.dma_start(prod_d[ic * 128:(ic + 1) * 128, :], psb[:])
                # h0's rel_x gather only needs THIS chunk - issue right away
                shear_one(0, ic, nc.sync, rx_only=True)

            for ic in range(4):
                emit_prod(ic)

            def conv_group2():
                grp, dst = 2, k_all
                for half in range(2):
                    ps = psB.tile([128, 1024], f32, tag="big", name="psb")[:, 0:512]
                    ps3 = ps[:].rearrange("p (a b) -> p a b", a=16)
                    for tap in range(9):
                        if half == 0 and tap in (1, 3, 5, 7):
                            emit_prod(4 + (tap - 1) // 2)
                        conv_mm(grp, half, ps3, [tap])
                    nc.scalar.activation(
                        dst[:, half * 512:(half + 1) * 512], ps[:],
                        AF.Identity, bias=ball_sb[:, grp:grp + 1])

            conv_group2()
            # remaining shear gathers AFTER every prod chunk is emitted (a
            # read emitted before its prod write would race: Tile orders by
            # program order, so the write would wait on the read instead).
            # rel_y reads transposed-pixel rows, so it needs ALL chunks.
            for ic in range(8):
                shear_one(0, ic, nc.sync, ry_only=True)
            shear_dmas(1)
            shear_dmas(3)
            # q/k/exey rows via DVE (32-partition pieces; sync stays free)
            def aug_rows(h, piece):
                if piece == 0:
                    nc.vector.tensor_copy(qaug[h][0:32, :],
                                          q_all[32 * h:32 * h + 32, :])
                elif piece == 1:
                    nc.vector.tensor_copy(kaug[h][0:32, :],
                                          k_all[32 * h:32 * h + 32, :])
                elif piece == 2:
                    nc.vector.tensor_copy(kaug[h][32:64, :], exey_sb[0:32, :])
                else:
                    nc.vector.tensor_copy(kaug[h][64:96, :], exey_sb[32:64, :])

            for piece in range(4):
                aug_rows(0, piece)
            conv_group(3, v_all)

            # v^T via PE transpose, then into 34-blocks (chunks 0-1 now,
            # the rest pipelined into head 0's jc loop)
            def vt_one(jc):
                tp = psT.tile([128, 128], bf16, tag="t")
                nc.tensor.transpose(tp[:], v_all[:, jc * 128:(jc + 1) * 128],
                                    identb[:])
                for hh in range(HPC):
                    nc.vector.tensor_copy(
                        vt_all[:, jc, 34 * hh:34 * hh + 32],
                        tp[:, 32 * hh:32 * hh + 32])

            for jc in range(2):
                vt_one(jc)

            # aug transposes: PE pair transpose -> 2 copies
            def aug_T(h, p, kind):
                src = (rxa if kind == 0 else rya)[h]
                rowbase = 32 + 32 * kind
                tp = psT.tile([128, 128], bf16, tag="t")
                nc.tensor.transpose(tp[0:64, :],
                                    src[:, 2 * p:2 * p + 2, :], identb[:])
                for k2 in range(2):
                    ic = 2 * p + k2
                    nc.vector.tensor_copy(
                        qaug[h][rowbase:rowbase + 32,
                                ic * 128:(ic + 1) * 128],
                        tp[32 * k2:32 * k2 + 32, :])

            for p in range(4):
                aug_T(0, p, 0)
                aug_T(0, p, 1)

            conv_group(0, None)
            shear_dmas(2)

            # ---------- head pipeline ----------
            def emit_ST(h, jc):
                ex = expp.tile([128, HW], bf16, tag="ex")
                st = psB.tile([128, HW], f32, tag="big")
                for nh_ in range(2):
                    sl = slice(nh_ * 512, (nh_ + 1) * 512)
                    nc.tensor.matmul(st[:, sl],
                                     kaug[h][:, jc * 128:(jc + 1) * 128],
                                     qaug[h][:, sl], start=True, stop=True)
                nc.scalar.activation(ex[:], st[:], AF.Exp, bias=negc[:])
                return ex

            def emit_AV(h, jc, ex, att_ps):
                for nh_ in range(2):
                    sl = slice(nh_ * 512, (nh_ + 1) * 512)
                    nc.tensor.matmul(att_ps[:, sl],
                                     vt_all[:, jc, 34 * h:34 * h + 33],
                                     ex[:, sl],
                                     start=(jc == 0), stop=(jc == 7))

            def fin_ic(h, ic):
                asb = attn_sb[h % 2]
                tp = psT.tile([128, 128], bf16, tag="t")
                nc.tensor.transpose(tp[:, 0:33],
                                    asb[:, ic * 128:(ic + 1) * 128],
                                    identb[0:33, 0:33])
                rec = work.tile([128, 1], f32, tag="rec")
                nc.vector.reciprocal(rec[:], tp[:, 32:33])
                nc.vector.tensor_tensor(att_hd[h % 2][:, ic, :],
                                        tp[:, 0:32],
                                        rec[:].to_broadcast((128, 32)), MUL)

            def fin_wr(h):
                engs = ([nc.gpsimd] * 8 if h < 3 else
                        [nc.sync, nc.sync, nc.sync, nc.gpsimd, nc.gpsimd,
                         nc.gpsimd, nc.scalar, nc.scalar])
                for ic in range(8):
                    dst = bass.AP(att_d, (32 * h + 4 * ic) * HW,
                                  [[HW, 4], [32, 32], [1, 32]])
                    engs[ic].dma_start(dst, att_hd[h % 2][:, ic, :])

            def fin_rd(h):
                nc.sync.dma_start(attr_sb[32 * h:32 * h + 32, :],
                                  att_d[32 * h:32 * h + 32, :])

            for h in range(HPC):
                att_ps = psB.tile([128, HW], f32, tag="big", name="avps")[0:33, :]
                exs = {}
                for jc in range(8):
                    exs[jc] = emit_ST(h, jc)
                    if jc >= 1:
                        emit_AV(h, jc - 1, exs.pop(jc - 1), att_ps)
                    if h == 0 and jc < 6:
                        vt_one(jc + 2)
                    if h + 1 < HPC and jc < 4:
                        aug_rows(h + 1, jc)
                    if h + 1 < HPC and jc >= 4:
                        aug_T(h + 1, jc - 4, 0)
                        aug_T(h + 1, jc - 4, 1)
                    if h > 0:
                        fin_ic(h - 1, jc)
                emit_AV(h, 7, exs.pop(7), att_ps)
                if h > 0:
                    fin_wr(h - 1)
                    fin_rd(h - 1)
                # att_ps -> sbuf; last head uses ACT (idle by then)
                if h == HPC - 1:
                    nc.scalar.activation(attn_sb[h % 2][:, 0:512],
                                         att_ps[:, 0:512], AF.Identity)
                    nc.scalar.activation(attn_sb[h % 2][:, 512:HW],
                                         att_ps[:, 512:HW], AF.Identity)
                else:
                    nc.vector.tensor_copy(attn_sb[h % 2][:, 0:512],
                                          att_ps[:, 0:512])
                    nc.vector.tensor_copy(attn_sb[h % 2][:, 512:HW],
                                          att_ps[:, 512:HW])

            for ic in range(8):
                fin_ic(3, ic)
            fin_wr(3)
            fin_rd(3)

            # ---------- 1x1 combine conv ----------
            for og in range(2):
                for half in range(2):
                    ps = psB.tile([128, 1024], f32, tag="big", name="psb")[:, 0:512]
                    nc.tensor.matmul(ps[:],
                                     awt_sb[:, og * 128:(og + 1) * 128],
                                     attr_sb[:, half * 512:(half + 1) * 512],
                                     start=True, stop=True)
                    oas = outp.tile([128, 512], f32, tag="oas")
                    nc.scalar.activation(oas[:], ps[:], AF.Identity,
                                         bias=ab_sb[:, og:og + 1])
                    eng = [nc.sync, nc.gpsimd, nc.scalar, nc.sync][og * 2 + half]
                    eng.dma_start(
                        oa_d[og * 128:(og + 1) * 128,
                             half * 512:(half + 1) * 512], oas[:])

    nc.finalize()
    return nc


def _prep_inputs(x, conv_w, conv_b, qkv_w, qkv_b, att_w, att_b, kr_x, kr_y):
    """Build the 8 per-core input maps (host-side numpy)."""
    sc = np.float32(DKH ** -0.5)
    krcat = np.concatenate([kr_x.T, kr_y.T], axis=1)        # (32, 126)
    krbd = np.zeros((128, PRODW), np.float32)
    for hh in range(HPC):
        krbd[32 * hh:32 * hh + 32, RELC * hh:RELC * (hh + 1)] = krcat
    exey = np.zeros((64, HW), np.float32)
    j = np.arange(HW)
    exey[j % W, j] = 1.0
    exey[32 + j // W, j] = 1.0

    def conv_lhsT(w):                                        # (co,ci,3,3)->(ci,9,co)
        return np.ascontiguousarray(w.transpose(1, 2, 3, 0).reshape(CIN, 9 * 128))

    in_maps = []
    for c in range(NCORES):
        b, g = divmod(c, 2)
        xp = np.zeros((CIN, PADW, PADW), np.float32)
        xp[:, 1:1 + H, 1:1 + W] = x[b]
        s = g * 128
        grps = [
            conv_lhsT(conv_w[s:s + 128]),
            conv_lhsT(qkv_w[s:s + 128] * sc),
            conv_lhsT(qkv_w[DK + s:DK + s + 128]),
            conv_lhsT(qkv_w[2 * DK + s:2 * DK + s + 128]),
        ]
        wall = np.stack(grps, axis=1).reshape(CIN, 4 * 9 * 128)
        ball = np.stack([
            conv_b[s:s + 128],
            qkv_b[s:s + 128] * sc,
            qkv_b[DK + s:DK + s + 128],
            qkv_b[2 * DK + s:2 * DK + s + 128],
        ]).astype(np.float32)
        awt = np.ascontiguousarray(att_w[:, s:s + 128, 0, 0].T)
        ab = (att_b.reshape(2, 128) if g == 0
              else np.zeros((2, 128))).astype(np.float32)
        in_maps.append({
            "xp": np.ascontiguousarray(xp.reshape(CIN, NPIX)).astype(BF),
            "wall": np.ascontiguousarray(wall).astype(BF),
            "ball": ball,
            "krbd": krbd.astype(BF),
            "exey": exey.astype(BF),
            "awt": awt.astype(BF),
            "ab": ab,
        })
    return in_maps


def kernel(x, conv_w, conv_b, qkv_w, qkv_b, att_w, att_b, kr_x, kr_y,
           _trace=False):
    from concourse.bass_utils import run_bass_kernel_spmd

    x = np.asarray(x, np.float32)
    conv_w = np.asarray(conv_w, np.float32)
    conv_b = np.asarray(conv_b, np.float32)
    qkv_w = np.asarray(qkv_w, np.float32)
    qkv_b = np.asarray(qkv_b, np.float32)
    att_w = np.asarray(att_w, np.float32)
    att_b = np.asarray(att_b, np.float32)
    kr_x = np.asarray(kr_x, np.float32)
    kr_y = np.asarray(kr_y, np.float32)

    if "nc" not in _CACHE:
        _CACHE["nc"] = _build_bass()
    nc = _CACHE["nc"]

    in_maps = _prep_inputs(x, conv_w, conv_b, qkv_w, qkv_b, att_w, att_b,
                           kr_x, kr_y)
    res = run_bass_kernel_spmd(nc, in_maps, core_ids=list(range(NCORES)),
                               trace=_trace)
    _CACHE["last_result"] = res

    out = np.empty((B, COUT, H, W), np.float32)
    for b in range(B):
        r0, r1 = res.results[2 * b], res.results[2 * b + 1]
        out[b, 0:128] = r0["oc"].reshape(128, H, W)
        out[b, 128:256] = r1["oc"].reshape(128, H, W)
        out[b, 256:512] = (r0["oa"] + r1["oa"]).reshape(256, H, W)
    return out
